# revision 29
# baseline (speedup 1.0000x reference)
"""DFND loss kernel for 8 TRN2 NeuronCores (Bass/Tile, SPMD).

Math (reference):
  pred   = argmax(preds_T, axis=1)                      # teacher label
  loss_t = logsumexp(preds_T) - max(preds_T)            # per-row CE at argmax
  sel    = k=N/2 rows with smallest loss_t (global)
  kl_i   = sum_c p_c (T_c - S_c) / sumT  + LS - LT      # p = softmax(T)
  loss   = sum_{sel} kl_i / N  +  mean_i( LS_i - log(dot(e^{S_i}, MT[pred_i])) - ... )
  where adapt = softmax(S) @ M, M = 0.95 I + 0.05 softmax-offdiag(noisy),
  and nll_i = log(sumS_i) - log(sum_c e^{S_ic} * MT[pred_i, c]).

Design:
  - Data-parallel over N: each of the 8 cores handles a contiguous block of
    rows.  Per-row stats (max/argmax/exp-sums/cross-dots) computed on device.
  - M^T built on device: each core softmaxes its slice of noisy_adaptation,
    assembles its M rows, transposes via TensorE, AllGathers the column
    blocks, and reassembles MT in its own DRAM.  MT rows are then gathered
    per-sample by pred via gpsimd dma_gather.
  - Global exact top-k threshold: per-row loss_t comes back to the host
    which computes the k-th order statistic (pure selection, no math), and a
    tiny second launch does the masked KL sum + NLL AllReduce on device.
"""

import numpy as np

N_CORES = 8
N = 16384
C = 1000
ROWS_PER_CORE = N // N_CORES  # 2048
K_SELECT = N // 2

_CACHE = {}


def _dt():
    from concourse import mybir

    return mybir.dt


def build_launch1(n_cores=N_CORES, rows=ROWS_PER_CORE, c_dim=C,
                  features=("par", "mtbuild", "gather")):
    """Per-core heavy pass. Returns (nc, names)."""
    import concourse.bacc as bacc
    import concourse.bass as bass
    import concourse.tile as tile
    from concourse import mybir
    import concourse.bass_isa as bass_isa

    dt = mybir.dt
    T = rows // 128  # row tiles per core
    assert rows % 128 == 0
    CH = 4  # tiles per gather chunk
    assert T % CH == 0 or T <= CH
    ch_tiles = min(CH, T)
    n_chunks = T // ch_tiles
    rows_per_chunk = ch_tiles * 128
    CP = 1024  # padded MT row length (bf16, 2048B, mult of 256B)
    nslice = 128  # na rows per core (padded)
    n_off = c_dim - 1

    nc = bacc.Bacc("TRN2", target_bir_lowering=False, debug=False,
                   num_devices=n_cores)
    f32, bf16 = dt.float32, dt.bfloat16

    pt_d = nc.dram_tensor("pt", [rows, c_dim], f32, kind="ExternalInput").ap()
    ps_d = nc.dram_tensor("ps", [rows, c_dim], f32, kind="ExternalInput").ap()
    na_d = nc.dram_tensor("na", [nslice, n_off], f32, kind="ExternalInput").ap()
    lmask_d = nc.dram_tensor("lmask", [nslice, n_off], f32, kind="ExternalInput").ap()
    dmask_d = nc.dram_tensor("dmask", [nslice, c_dim], f32, kind="ExternalInput").ap()
    ident_d = nc.dram_tensor("ident", [128, 128], bf16, kind="ExternalInput").ap()

    losst_d = nc.dram_tensor("loss_t", [128, T], f32, kind="ExternalOutput").ap()
    kl_d = nc.dram_tensor("klrow", [128, T], f32, kind="ExternalOutput").ap()
    nll_d = nc.dram_tensor("nllsum", [1, 1], f32, kind="ExternalOutput").ap()
    import os
    dump_mt = bool(os.environ.get("KERNEL_DUMP_MT")) and "mtbuild" in features
    if dump_mt:
        mtdump_d = nc.dram_tensor("mtdump", [1024, 1024], dt.bfloat16,
                                  kind="ExternalOutput").ap()
        preddump_d = nc.dram_tensor("preddump", [128, T], dt.int16,
                                    kind="ExternalOutput").ap()
        gdump_d = nc.dram_tensor("gdump", [128, T], f32,
                                 kind="ExternalOutput").ap()

    with tile.TileContext(nc) as tc:
        with (
            tc.tile_pool(name="singles", bufs=1) as singles,
            tc.tile_pool(name="ld", bufs=3) as ld,
            tc.tile_pool(name="exps", bufs=2 + ch_tiles) as exps,
            tc.tile_pool(name="tmp", bufs=2) as tmp,
            tc.tile_pool(name="mb", bufs=1) as mb,
            tc.tile_pool(name="psum", bufs=2, space="PSUM") as psum,
            tc.tile_pool(name="dramp", bufs=1, space="DRAM") as dramp,
        ):
            # internal DRAM
            mt_d = dramp.tile([CP, CP], bf16)            # MT rows (padded)
            scr_d = dramp.tile([rows], dt.int16)         # pred scratch, row order
            contrib_d = dramp.tile([8 * 128 * 128], bf16)
            ag_d = dramp.tile([n_cores * 8 * 128 * 128], bf16)
            # ---- per-row stat buffers (partition p, tile t) = row t*128+p
            sumT = singles.tile([128, T], f32)
            sumS = singles.tile([128, T], f32)
            wT = singles.tile([128, T], f32)
            gdot = singles.tile([128, T], f32)
            maxT = singles.tile([128, T], f32)
            pred16 = singles.tile([128, T], dt.int16)
            idxw = singles.tile([128, 8 * T], dt.int16)   # wrapped-16 gather idxs
            mrows = singles.tile([128, T, CP], bf16)      # gathered MT rows

            nc.vector.memset(idxw, 0)
            if "gather" not in features:
                nc.vector.memset(gdot, 1.0)

            # ================= MT build =================
            if "mtbuild" in features:
                # softmax of this core's noisy_adaptation slice
                na_t = mb.tile([nslice, n_off], f32)
                nc.sync.dma_start(out=na_t, in_=na_d)
                eN = mb.tile([nslice, n_off], f32)
                sumN = mb.tile([nslice, 1], f32)
                nc.scalar.activation(out=eN, in_=na_t,
                                     func=mybir.ActivationFunctionType.Exp,
                                     accum_out=sumN)
                rN = mb.tile([nslice, 1], f32)
                nc.vector.reciprocal(out=rN, in_=sumN)
                soft = mb.tile([nslice, n_off], f32)
                # soft = eN * (0.05 / sumN)
                nc.vector.tensor_scalar(out=soft, in0=eN, scalar1=rN,
                                        scalar2=0.05,
                                        op0=mybir.AluOpType.mult,
                                        op1=mybir.AluOpType.mult)
                lm = mb.tile([nslice, n_off], f32)
                nc.sync.dma_start(out=lm, in_=lmask_d)
                dm = mb.tile([nslice, c_dim], f32)
                nc.sync.dma_start(out=dm, in_=dmask_d)
                # a = soft * lmask ; b = soft - a
                a_t = mb.tile([nslice, n_off], f32)
                nc.vector.tensor_tensor(out=a_t, in0=soft, in1=lm,
                                        op=mybir.AluOpType.mult)
                b_t = mb.tile([nslice, n_off], f32)
                nc.vector.tensor_tensor(out=b_t, in0=soft, in1=a_t,
                                        op=mybir.AluOpType.subtract)
                # M = [a ; 0] + [0 ; b] + dmask(0.95 at diag)
                m_t = mb.tile([nslice, c_dim], f32)
                nc.scalar.copy(out=m_t[:, 0:n_off], in_=a_t)
                nc.vector.memset(m_t[:, n_off:c_dim], 0.0)
                nc.vector.tensor_tensor(out=m_t[:, 1:c_dim],
                                        in0=m_t[:, 1:c_dim],
                                        in1=b_t, op=mybir.AluOpType.add)
                nc.vector.tensor_tensor(out=m_t, in0=m_t, in1=dm,
                                        op=mybir.AluOpType.add)
                mbf = mb.tile([nslice, c_dim], bf16)
                nc.scalar.copy(out=mbf, in_=m_t)
                ident = singles.tile([128, 128], bf16)
                nc.sync.dma_start(out=ident, in_=ident_d)
                # transpose column chunks -> contrib blocks [u][q][ci]
                n_ch = (c_dim + 127) // 128
                for u in range(8):
                    if u < n_ch:
                        w = min(128, c_dim - u * 128)
                        pt_ps = psum.tile([128, 128], bf16, tag="tp")
                        nc.tensor.transpose(out=pt_ps[0:w, 0:nslice],
                                            in_=mbf[:, u * 128:u * 128 + w],
                                            identity=ident)
                        cb = tmp.tile([128, 128], bf16, tag="cb")
                        if w < 128:
                            nc.vector.memset(cb, 0.0)
                        nc.scalar.copy(out=cb[0:w, :], in_=pt_ps[0:w, 0:nslice])
                        nc.sync.dma_start(
                            out=contrib_d[u * 128 * 128:(u + 1) * 128 * 128]
                            .rearrange("(q x) -> q x", q=128),
                            in_=cb)
                    else:
                        cb = tmp.tile([128, 128], bf16, tag="cb")
                        nc.vector.memset(cb, 0.0)
                        nc.sync.dma_start(
                            out=contrib_d[u * 128 * 128:(u + 1) * 128 * 128]
                            .rearrange("(q x) -> q x", q=128),
                            in_=cb)
                nc.gpsimd.collective_compute(
                    "AllGather", mybir.AluOpType.bypass,
                    replica_groups=[list(range(n_cores))],
                    ins=[contrib_d.opt()], outs=[ag_d.opt()])
                # reassemble: mt[u*128+q, c*128+ci] = ag[c][u][q][ci]
                src_v = ag_d.rearrange("(c u q x) -> c u q x", c=n_cores,
                                       u=8, q=128)
                dst_v = mt_d.rearrange("(u q) (c x) -> c u q x", u=8, c=8)
                for cc in range(n_cores):
                    nc.sync.dma_start(out=dst_v[cc], in_=src_v[cc])
                if dump_mt:
                    nc.sync.dma_start(out=mtdump_d, in_=mt_d)

            # ================= main loop =================
            eSs = []
            for t in range(T):
                pt_t = ld.tile([128, c_dim], f32, tag="pt")
                nc.sync.dma_start(out=pt_t, in_=pt_d[t * 128:(t + 1) * 128, :])
                ps_t = ld.tile([128, c_dim], f32, tag="ps")
                nc.sync.dma_start(out=ps_t, in_=ps_d[t * 128:(t + 1) * 128, :])

                eT_t = tmp.tile([128, c_dim], bf16, tag="eT")
                nc.scalar.activation(out=eT_t, in_=pt_t,
                                     func=mybir.ActivationFunctionType.Exp,
                                     accum_out=sumT[:, t:t + 1])
                eS_t = exps.tile([128, c_dim], bf16, tag="eS")
                eSs.append(eS_t)
                nc.scalar.activation(out=eS_t, in_=ps_t,
                                     func=mybir.ActivationFunctionType.Exp,
                                     accum_out=sumS[:, t:t + 1])

                mx8 = tmp.tile([128, 8], f32, tag="mx8")
                nc.vector.max(out=mx8, in_=pt_t)
                ix8 = tmp.tile([128, 8], dt.uint16, tag="ix8")
                nc.vector.max_index(out=ix8, in_max=mx8, in_values=pt_t)
                nc.gpsimd.tensor_copy(out=maxT[:, t:t + 1], in_=mx8[:, 0:1])
                nc.gpsimd.tensor_copy(out=pred16[:, t:t + 1], in_=ix8[:, 0:1])

                d_t = tmp.tile([128, c_dim], bf16, tag="d")
                nc.vector.tensor_tensor(out=d_t, in0=pt_t, in1=ps_t,
                                        op=mybir.AluOpType.subtract)
                pw_t = tmp.tile([128, c_dim], bf16, tag="pw")
                nc.vector.tensor_tensor(out=pw_t, in0=eT_t, in1=d_t,
                                        op=mybir.AluOpType.mult)
                nc.vector.tensor_reduce(out=wT[:, t:t + 1], in_=pw_t,
                                        axis=mybir.AxisListType.X,
                                        op=mybir.AluOpType.add)

                if (t + 1) % ch_tiles == 0 and "gather" in features:
                    ch = t // ch_tiles
                    t0 = ch * ch_tiles
                    # pred16[:, t0:t0+ch] -> scr (row order) -> idxw (wrap16)
                    nc.sync.dma_start(
                        out=scr_d.rearrange("(t p) -> p t", p=128)[:, t0:t0 + ch_tiles],
                        in_=pred16[:, t0:t0 + ch_tiles])
                    jslots = 8 * ch_tiles
                    # replicate the wrapped idx list into all 8 Q7-core
                    # partition groups (each core reads its own 16 partitions)
                    src_w = scr_d.rearrange("(j q) -> q j", q=16)[
                        :, ch * jslots:(ch + 1) * jslots]
                    for g in range(8):
                        nc.sync.dma_start(
                            out=idxw[g * 16:(g + 1) * 16,
                                     ch * jslots:(ch + 1) * jslots],
                            in_=src_w)
                    nc.gpsimd.dma_gather(
                        out_ap=mrows[:, t0:t0 + ch_tiles, :],
                        in_ap=mt_d,
                        idxs_ap=idxw[:, ch * jslots:(ch + 1) * jslots],
                        num_idxs=rows_per_chunk,
                        num_idxs_reg=rows_per_chunk,
                        elem_size=CP)
                    for tt in range(t0, t0 + ch_tiles):
                        pg_t = tmp.tile([128, c_dim], bf16, tag="pg")
                        nc.vector.tensor_tensor(out=pg_t, in0=eSs[tt],
                                                in1=mrows[:, tt, 0:c_dim],
                                                op=mybir.AluOpType.mult)
                        nc.vector.tensor_reduce(out=gdot[:, tt:tt + 1],
                                                in_=pg_t,
                                                axis=mybir.AxisListType.X,
                                                op=mybir.AluOpType.add)

            # ================= finishing =================
            LT = singles.tile([128, T], f32)
            nc.scalar.activation(out=LT, in_=sumT,
                                 func=mybir.ActivationFunctionType.Ln)
            LS = singles.tile([128, T], f32)
            nc.scalar.activation(out=LS, in_=sumS,
                                 func=mybir.ActivationFunctionType.Ln)
            Ld = singles.tile([128, T], f32)
            nc.scalar.activation(out=Ld, in_=gdot,
                                 func=mybir.ActivationFunctionType.Ln)

            lt_b = singles.tile([128, T], f32)
            nc.vector.tensor_tensor(out=lt_b, in0=LT, in1=maxT,
                                    op=mybir.AluOpType.subtract)
            nc.sync.dma_start(out=losst_d, in_=lt_b)

            rT = singles.tile([128, T], f32)
            nc.vector.reciprocal(out=rT, in_=sumT)
            kl_b = singles.tile([128, T], f32)
            nc.vector.tensor_tensor(out=kl_b, in0=wT, in1=rT,
                                    op=mybir.AluOpType.mult)
            dLST = singles.tile([128, T], f32)
            nc.vector.tensor_tensor(out=dLST, in0=LS, in1=LT,
                                    op=mybir.AluOpType.subtract)
            nc.vector.tensor_tensor(out=kl_b, in0=kl_b, in1=dLST,
                                    op=mybir.AluOpType.add)
            nc.sync.dma_start(out=kl_d, in_=kl_b)

            nll_b = singles.tile([128, T], f32)
            nc.vector.tensor_tensor(out=nll_b, in0=LS, in1=Ld,
                                    op=mybir.AluOpType.subtract)
            if dump_mt:
                nc.sync.dma_start(out=preddump_d, in_=pred16)
                nc.sync.dma_start(out=gdump_d, in_=gdot)
            nll_r = singles.tile([128, 1], f32)
            nc.vector.tensor_reduce(out=nll_r, in_=nll_b,
                                    axis=mybir.AxisListType.X,
                                    op=mybir.AluOpType.add)
            if "par" in features:
                nll_a = singles.tile([128, 1], f32)
                nc.gpsimd.partition_all_reduce(out_ap=nll_a, in_ap=nll_r,
                                               channels=128,
                                               reduce_op=bass_isa.ReduceOp.add)
                nc.sync.dma_start(out=nll_d, in_=nll_a[0:1, 0:1])
            else:
                nc.sync.dma_start(out=nll_d, in_=nll_r[0:1, 0:1])

    nc.compile()
    return nc


def build_launch2(n_cores=N_CORES, rows=ROWS_PER_CORE):
    import concourse.bacc as bacc
    import concourse.tile as tile
    from concourse import mybir
    import concourse.bass_isa as bass_isa

    dt = mybir.dt
    T = rows // 128
    f32 = dt.float32
    nc = bacc.Bacc("TRN2", target_bir_lowering=False, debug=False,
                   num_devices=n_cores)
    kl_d = nc.dram_tensor("klrow", [128, T], f32, kind="ExternalInput").ap()
    mask_d = nc.dram_tensor("mask", [128, T], f32, kind="ExternalInput").ap()
    nllp_d = nc.dram_tensor("nllp", [1, 1], f32, kind="ExternalInput").ap()
    loss_d = nc.dram_tensor("loss", [1, 1], f32, kind="ExternalOutput").ap()

    with tile.TileContext(nc) as tc:
        with (
            tc.tile_pool(name="sb", bufs=1) as sb,
            tc.tile_pool(name="dram", bufs=1, space="DRAM") as dram,
        ):
            kl_t = sb.tile([128, T], f32)
            nc.sync.dma_start(out=kl_t, in_=kl_d)
            mk_t = sb.tile([128, T], f32)
            nc.sync.dma_start(out=mk_t, in_=mask_d)
            np_t = sb.tile([1, 1], f32)
            nc.sync.dma_start(out=np_t, in_=nllp_d)
            junk = sb.tile([128, T], f32)
            nc.vector.tensor_tensor(out=junk, in0=kl_t, in1=mk_t,
                                    op=mybir.AluOpType.mult)
            acc = sb.tile([128, 1], f32)
            nc.vector.tensor_reduce(out=acc, in_=junk,
                                    axis=mybir.AxisListType.X,
                                    op=mybir.AluOpType.add)
            allp = sb.tile([128, 1], f32)
            nc.gpsimd.partition_all_reduce(out_ap=allp, in_ap=acc,
                                           channels=128,
                                           reduce_op=bass_isa.ReduceOp.add)
            tot = sb.tile([1, 1], f32)
            nc.vector.tensor_tensor(out=tot, in0=allp[0:1, 0:1], in1=np_t,
                                    op=mybir.AluOpType.add)
            cin = dram.tile([1, 1], f32)
            cout = dram.tile([1, 1], f32)
            nc.gpsimd.dma_start(out=cin[:], in_=tot)
            nc.gpsimd.collective_compute(
                "AllReduce", mybir.AluOpType.add,
                replica_groups=[list(range(n_cores))],
                ins=[cin.opt()], outs=[cout.opt()])
            res = sb.tile([1, 1], f32)
            nc.gpsimd.dma_start(out=res, in_=cout[:])
            out_t = sb.tile([1, 1], f32)
            nc.scalar.mul(out=out_t, in_=res, mul=1.0 / (rows * n_cores))
            nc.sync.dma_start(out=loss_d, in_=out_t)

    nc.compile()
    return nc


def host_constants(n_cores=N_CORES, c_dim=C):
    """Per-core index-structure constants (masks, identity)."""
    n_off = c_dim - 1
    nslice = 128
    consts = []
    for c in range(n_cores):
        r0 = c * nslice
        ig = r0 + np.arange(nslice)[:, None]          # global M-row per partition
        j = np.arange(n_off)[None, :]
        lmask = (j < ig).astype(np.float32)           # soft col j used for M col j when j < ig
        jc = np.arange(c_dim)[None, :]
        dmask = np.where(jc == ig, np.float32(0.95), np.float32(0.0))
        valid = (ig < c_dim)
        lmask = lmask * valid
        dmask = dmask * valid
        consts.append({"lmask": lmask.astype(np.float32),
                       "dmask": dmask.astype(np.float32)})
    ident = np.eye(128, dtype=np.float32)
    try:
        import ml_dtypes
        ident_bf = ident.astype(ml_dtypes.bfloat16)
    except ImportError:
        ident_bf = ident
    for c in consts:
        c["ident"] = ident_bf
    return consts


def _unshuffle(arr):
    """(128, T) [p, t] -> (128*T,) row order r = t*128 + p."""
    return np.ascontiguousarray(arr.T).ravel()


def _make_runner(nc, n_cores=N_CORES):
    """Build a cached jitted SPMD callable for a compiled Bacc program.

    Mirrors bass2jax.run_bass_via_pjrt but constructs the jit once so
    repeated kernel() calls skip retracing.
    """
    import jax
    import numpy as _np
    from jax.sharding import Mesh, PartitionSpec, NamedSharding
    from jax.experimental.shard_map import shard_map
    from concourse import mybir as mb
    from concourse.bass2jax import (_bass_exec_p, partition_id_tensor,
                                    install_neuronx_cc_hook)

    install_neuronx_cc_hook()
    partition_name = (nc.partition_id_tensor.name
                      if nc.partition_id_tensor else None)
    in_names, out_names, out_avals, zero_outs = [], [], [], []
    for alloc in nc.m.functions[0].allocations:
        if not isinstance(alloc, mb.MemoryLocationSet):
            continue
        name = alloc.memorylocations[0].name
        if alloc.kind == "ExternalInput":
            if name != partition_name:
                in_names.append(name)
        elif alloc.kind == "ExternalOutput":
            out_names.append(name)
            shape = tuple(alloc.tensor_shape)
            dtype = mb.dt.np(alloc.dtype)
            out_avals.append(jax.core.ShapedArray(shape, dtype))
            zero_outs.append(_np.zeros(shape, dtype))
    n_params = len(in_names)
    param_names = list(in_names)
    in_names = in_names + out_names
    if partition_name is not None:
        in_names.append(partition_name)

    def _body(*args):
        operands = list(args)
        if partition_name is not None:
            operands.append(partition_id_tensor())
        outs = _bass_exec_p.bind(
            *operands, out_avals=tuple(out_avals), in_names=tuple(in_names),
            out_names=tuple(out_names), lowering_input_output_aliases=(),
            sim_require_finite=True, sim_require_nnan=True, nc=nc)
        return tuple(outs)

    devices = jax.devices()[:n_cores]
    mesh = Mesh(_np.asarray(devices), ("core",))
    nspecs = n_params + len(out_names)
    fn = jax.jit(
        shard_map(_body, mesh=mesh,
                  in_specs=(PartitionSpec("core"),) * nspecs,
                  out_specs=(PartitionSpec("core"),) * len(out_names),
                  check_rep=False),
        keep_unused=True)
    sharding = NamedSharding(mesh, PartitionSpec("core"))
    concat_zeros = [
        _np.zeros((n_cores * z.shape[0], *z.shape[1:]), z.dtype)
        for z in zero_outs]

    def run(in_maps, device_args=None):
        if device_args is None:
            device_args = [
                _np.concatenate([_np.asarray(in_maps[c][k])
                                 for c in range(n_cores)], axis=0)
                for k in param_names]
        out_arrs = fn(*device_args, *concat_zeros)
        out_arrs = [_np.asarray(o) for o in out_arrs]
        return [
            {name: out_arrs[i].reshape(n_cores, *out_avals[i].shape)[c]
             for i, name in enumerate(out_names)}
            for c in range(n_cores)
        ]

    run.param_names = param_names
    run.sharding = sharding
    run.fn = fn
    run.concat_zeros = concat_zeros
    return run


def kernel(preds_S, preds_T, noisy_adaptation):
    if "nc1" not in _CACHE:
        _CACHE["nc1"] = build_launch1()
        _CACHE["nc2"] = build_launch2()
        _CACHE["consts"] = host_constants()
        _CACHE["run1"] = _make_runner(_CACHE["nc1"])
        _CACHE["run2"] = _make_runner(_CACHE["nc2"])
    nc1, nc2, consts = _CACHE["nc1"], _CACHE["nc2"], _CACHE["consts"]
    run1, run2 = _CACHE["run1"], _CACHE["run2"]

    preds_S = np.asarray(preds_S, dtype=np.float32)
    preds_T = np.asarray(preds_T, dtype=np.float32)
    noisy = np.asarray(noisy_adaptation, dtype=np.float32)

    R = ROWS_PER_CORE
    na_pad = np.zeros((N_CORES * 128, C - 1), np.float32)
    na_pad[:C] = noisy
    in_maps = []
    for c in range(N_CORES):
        in_maps.append({
            "pt": preds_T[c * R:(c + 1) * R],
            "ps": preds_S[c * R:(c + 1) * R],
            "na": na_pad[c * 128:(c + 1) * 128],
            **consts[c],
        })
    import os
    _dbg = os.environ.get("KERNEL_DEBUG")
    if _dbg:
        print("[kernel] running launch1...", flush=True)
    res1 = run1(in_maps)
    if _dbg:
        print("[kernel] launch1 done", flush=True)

    loss_t = np.concatenate([_unshuffle(res1[c]["loss_t"])
                             for c in range(N_CORES)])
    # exact global top-k selection (smallest loss_t), value+index order like
    # lax.top_k on -loss_t
    order = np.argsort(loss_t, kind="stable")
    sel = np.zeros(N, np.float32)
    sel[order[:K_SELECT]] = 1.0

    in_maps2 = []
    for c in range(N_CORES):
        m = sel[c * R:(c + 1) * R].reshape(ROWS_PER_CORE // 128, 128).T
        in_maps2.append({
            "klrow": res1[c]["klrow"],
            "mask": np.ascontiguousarray(m),
            "nllp": res1[c]["nllsum"],
        })
    if _dbg:
        print("[kernel] running launch2...", flush=True)
    res2 = run2(in_maps2)
    if _dbg:
        print("[kernel] launch2 done", flush=True)
    loss = res2[0]["loss"][0, 0]
    return np.float32(loss)
